# revision 2
# baseline (speedup 1.0000x reference)
"""Trainium2 Bass kernel for the ragged per-layer decoder stack.

out[b, i, a] = sum_{j<=i} sum_f x[b, j, f] * W[i, j, f, a]
  x: [256, 12, 2048] f32,  W: [12, 12, 2048, 768] f32 -> out: [256, 12, 768] f32

Sharding: W's d_features axis (F=2048) is split across the 8 NeuronCores
(256 features each). Each core contracts its feature slice against the
lower-triangular (j<=i) weight blocks and produces a full partial output;
the host sums the 8 partials (the all-reduce) and transposes back to
[256, 12, 768].

Matmuls run in bf16 (hostside cast) with fp32 PSUM accumulation.
Weight DMAs are (j,k)-merged per layer i and partition-major packed so
every partition row is one long contiguous run (>=4KB descriptors
saturate the HBM bus: measured 345 GB/s); one DMA per i (12 total)
minimizes per-transfer DGE overhead. Output tiles pack both batch tiles
side by side ([128, 2*768]) so each layer writes one out-DMA (12 total)
on the ACT HWDGE ring, where its semaphore waits cannot head-of-line-
block the W stream on the SP ring; the host unpermutes the batch
interleave for free. PSUM accumulation runs k-major so each group can
start as soon as its W tile lands.
"""

import numpy as np
import ml_dtypes

import concourse.bass as bass
import concourse.tile as tile
from concourse import bacc, mybir
from concourse.bass_utils import run_bass_kernel_spmd

BF16 = ml_dtypes.bfloat16

# Problem shape (hardcoded per contract)
B = 256      # batch
L = 12       # layers
F = 2048     # d_features
A = 768      # d_activations
NCORES = 8
FC = F // NCORES      # feature slice per core = 256
P = 128               # partitions
NK = FC // P          # k-tiles per core slice = 2
NB = B // P           # batch tiles = 2
AC = 384              # activation chunk per matmul (2 chunks of 384 <= 512 PSUM)
NPAIR = sum(i + 1 for i in range(L)) * NK   # 156 weight tiles per core

_PAIRS = [(i, j) for i in range(L) for j in range(i + 1)]

# --- tuning knobs (affect build_module; set before first call) ---
WBUFS = 4         # W block pool slots (each sized [128, 2*12*768] bf16)
OBUFS = 4         # output tile pool slots
PSBUFS = 8        # PSUM pool slots (banks)
HWLOOP = True     # use tc.For_i for repeat>1 (bench only)

# W block (i, k) tile offset in wpack: tiles [j=0..i] for fixed k
_WBASE = {}
_off = 0
for _i in range(L):
    for _k in range(NK):
        _WBASE[(_i, _k)] = _off
        _off += _i + 1
assert _off == NPAIR


def _emit_kernel(ctx, tc, xpack, wpack, out, repeat=1):
    nc = tc.nc
    xpool = ctx.enter_context(tc.tile_pool(name="xpool", bufs=1))
    wpool = ctx.enter_context(tc.tile_pool(name="wpool", bufs=WBUFS))
    opool = ctx.enter_context(tc.tile_pool(name="opool", bufs=OBUFS))
    pspool = ctx.enter_context(tc.tile_pool(name="pspool", bufs=PSBUFS, space="PSUM"))

    # x resident in SBUF for the whole kernel, single merged tile:
    # xt[p, (k*L + j)*B + b] = x[b, j, c*FC + k*P + p]
    xt = xpool.tile([P, NK * L * B], mybir.dt.bfloat16, tag="x")
    nc.sync.dma_start(xt[:], xpack[:])

    if repeat > 1 and HWLOOP:
        with tc.For_i(0, repeat, 1, hint_engines=(
                mybir.EngineType.PE, mybir.EngineType.SP)):
            _emit_body(tc, xt, wpack, out, wpool, opool, pspool)
    else:
        for _ in range(repeat):
            _emit_body(tc, xt, wpack, out, wpool, opool, pspool)


def _emit_body(tc, xt, wpack, out, wpool, opool, pspool):
    nc = tc.nc
    for i in range(L):
        n = i + 1
        # One merged weight DMA per i covering both k tiles (adjacent in
        # wpack): [128, 2*n*768] bf16. wpack is partition-major, so each
        # partition row is one contiguous 2*n*1536B run.
        wt = wpool.tile([P, NK * n * A], mybir.dt.bfloat16, tag="w")
        base = _WBASE[(i, 0)] * A
        nc.sync.dma_start(wt[:], wpack[:, base:base + NK * n * A])
        # within wt, block (k, j) occupies columns [(k*n + j)*A : +A]
        jks = [(j, k) for k in range(NK) for j in range(n)]   # k-major
        acs = [(0, AC), (AC, AC)]
        ot = opool.tile([P, NB * A], mybir.dt.bfloat16)
        for bt in range(NB):
            pss = [pspool.tile([P, w], mybir.dt.float32, name=f"ps{ci}",
                               tag=f"ps{ci}", bufs=PSBUFS // 2)
                   for ci, (_, w) in enumerate(acs)]
            for ps, (off, w) in zip(pss, acs):
                for t, (j, k) in enumerate(jks):
                    nc.tensor.matmul(
                        ps[:],
                        xt[:, (k * L + j) * B + bt * P:(k * L + j) * B + bt * P + P],
                        wt[:, (k * n + j) * A + off:(k * n + j) * A + off + w],
                        start=(t == 0), stop=(t == len(jks) - 1),
                    )
            nc.vector.tensor_copy(ot[:, bt * A:bt * A + acs[0][1]], pss[0][:])
            nc.vector.tensor_copy(ot[:, bt * A + acs[0][1]:(bt + 1) * A], pss[1][:])
        # single out-DMA per i on the ACT HWDGE ring; out[i] holds both
        # batch tiles side by side: out[i, p, bt*A + a] = batch bt*128+p
        nc.scalar.dma_start(out[i, :, :], ot[:])


_NC_CACHE = {}


def build_module(repeat=1):
    key = (repeat, WBUFS, OBUFS, PSBUFS, HWLOOP)
    if key in _NC_CACHE:
        return _NC_CACHE[key]
    from contextlib import ExitStack
    nc = bacc.Bacc(
        "TRN2",
        target_bir_lowering=False,
        debug=False,
        enable_asserts=False,
        num_devices=NCORES,
    )
    xpack = nc.dram_tensor(
        "xpack", [P, NK * L * B], mybir.dt.bfloat16, kind="ExternalInput").ap()
    wpack = nc.dram_tensor(
        "wpack", [P, NPAIR * A], mybir.dt.bfloat16, kind="ExternalInput").ap()
    out = nc.dram_tensor(
        "out", [L, P, NB * A], mybir.dt.bfloat16, kind="ExternalOutput").ap()
    with tile.TileContext(nc) as tc:
        with ExitStack() as ctx:
            _emit_kernel(ctx, tc, xpack, wpack, out, repeat=repeat)
    nc.compile()
    _NC_CACHE[key] = nc
    return nc


def prep_inputs(x, W):
    """Build per-core packed inputs. Returns (xpacks[8], wpacks[8])."""
    # xpack[c][p, (k*L + j)*B + b] = x[b, j, c*FC + k*P + p]
    xb = np.asarray(x, dtype=BF16)                       # [256, 12, 2048]
    xr = xb.reshape(B, L, NCORES, NK, P).transpose(2, 4, 3, 1, 0)
    xpacks = np.ascontiguousarray(xr).reshape(NCORES, P, NK * L * B)

    # wpack[c]: partition-major; per (i, k) block occupies free columns
    # [_WBASE*A : (_WBASE+n)*A], j inner:
    #   wpack[c][p, (_WBASE[(i,k)] + j)*A + a] = W[i, j, c*FC + k*P + p, a]
    Ii = [i for i, j in _PAIRS]
    Jj = [j for i, j in _PAIRS]
    Wtri = np.asarray(W, dtype=BF16)[Ii, Jj]             # [78, 2048, 768]
    Wtri = Wtri.reshape(len(_PAIRS), NCORES, NK, P, A)   # [78, c, k, p, a]
    pidx = {}
    for t, (i, j) in enumerate(_PAIRS):
        pidx[(i, j)] = t
    sel_pair, sel_k = [], []
    for i in range(L):
        for k in range(NK):
            for j in range(i + 1):
                sel_pair.append(pidx[(i, j)])
                sel_k.append(k)
    Wp = Wtri[sel_pair, :, sel_k]                        # [156, c, 128, 768]
    Wp = np.ascontiguousarray(Wp.transpose(1, 2, 0, 3))  # [c, p, 156, a]
    wpacks = Wp.reshape(NCORES, P, NPAIR * A)
    return xpacks, wpacks


def run(x, W, trace=False, **kw):
    """Run the SPMD kernel; returns (full_output, BassKernelResults)."""
    x = np.asarray(x, dtype=np.float32)
    W = np.asarray(W, dtype=np.float32)
    xpacks, wpacks = prep_inputs(x, W)
    nc = build_module()
    in_maps = [{"xpack": xpacks[c], "wpack": wpacks[c]} for c in range(NCORES)]
    res = run_bass_kernel_spmd(nc, in_maps, list(range(NCORES)), trace=trace, **kw)
    total = res.results[0]["out"].astype(np.float32)
    for c in range(1, NCORES):
        total = total + res.results[c]["out"].astype(np.float32)
    # out[i, p, bt*A + a] = partial for batch bt*128+p -> [L, B, A]
    total = total.reshape(L, P, NB, A).transpose(0, 2, 1, 3).reshape(L, B, A)
    full = np.ascontiguousarray(total.transpose(1, 0, 2))
    return full, res


def kernel(x, W):
    full, _ = run(x, W)
    return full


# revision 6
# speedup vs baseline: 1.0283x; 1.0283x over previous
"""Trainium2 Bass kernel for the ragged per-layer decoder stack.

out[b, i, a] = sum_{j<=i} sum_f x[b, j, f] * W[i, j, f, a]
  x: [256, 12, 2048] f32,  W: [12, 12, 2048, 768] f32 -> out: [256, 12, 768] f32

Sharding: W's d_features axis (F=2048) is split across the 8 NeuronCores
(256 features each). Each core contracts its feature slice against the
lower-triangular (j<=i) weight blocks and produces a full partial output;
the host sums the 8 partials (the all-reduce) and transposes back to
[256, 12, 768].

Matmuls run in bf16 (hostside cast) with fp32 PSUM accumulation.
Weight DMAs are (j,k)-merged per layer i and partition-major packed so
every partition row is one long contiguous run (>=4KB descriptors
saturate the HBM bus: measured 345 GB/s); one DMA per i (12 total)
minimizes per-transfer DGE overhead. Output tiles pack both batch tiles
side by side ([128, 2*768]) so each layer writes one out-DMA (12 total)
on the ACT HWDGE ring, where its semaphore waits cannot head-of-line-
block the W stream on the SP ring; the host unpermutes the batch
interleave for free. PSUM accumulation runs k-major so each group can
start as soon as its W tile lands.
"""

import numpy as np
import ml_dtypes

import concourse.bass as bass
import concourse.tile as tile
from concourse import bacc, mybir
from concourse.bass_utils import run_bass_kernel_spmd

BF16 = ml_dtypes.bfloat16

# Problem shape (hardcoded per contract)
B = 256      # batch
L = 12       # layers
F = 2048     # d_features
A = 768      # d_activations
NCORES = 8
FC = F // NCORES      # feature slice per core = 256
P = 128               # partitions
NK = FC // P          # k-tiles per core slice = 2
NB = B // P           # batch tiles = 2
AC = 384              # activation chunk per matmul (2 chunks of 384 <= 512 PSUM)
NPAIR = sum(i + 1 for i in range(L)) * NK   # 156 weight tiles per core

_PAIRS = [(i, j) for i in range(L) for j in range(i + 1)]

# --- tuning knobs (affect build_module; set before first call) ---
WBUFS = 4         # W block pool slots (each sized [128, 2*12*768] bf16)
OBUFS = 4         # output tile pool slots
PSBUFS = 8        # PSUM pool slots (banks)
HWLOOP = True     # use tc.For_i for repeat>1 (bench only)
UNROLL = 1        # bodies per For_i iteration. 2 was tried to amortize the
                  # loop back-edge, but the multi-body For_i + remainder
                  # structure produced a NEFF that crashed the exec unit
                  # (NRT_EXEC_UNIT_UNRECOVERABLE); keep 1.

# W block (i, k) tile offset in wpack: tiles [j=0..i] for fixed k
_WBASE = {}
_off = 0
for _i in range(L):
    for _k in range(NK):
        _WBASE[(_i, _k)] = _off
        _off += _i + 1
assert _off == NPAIR


def _emit_kernel(ctx, tc, xpack, wpack, out, repeat=1):
    nc = tc.nc
    xpool = ctx.enter_context(tc.tile_pool(name="xpool", bufs=1))
    wpool = ctx.enter_context(tc.tile_pool(name="wpool", bufs=WBUFS))
    opool = ctx.enter_context(tc.tile_pool(name="opool", bufs=OBUFS))
    pspool = ctx.enter_context(tc.tile_pool(name="pspool", bufs=PSBUFS, space="PSUM"))

    # x resident in SBUF for the whole kernel, single merged tile:
    # xt[p, (k*L + j)*B + b] = x[b, j, c*FC + k*P + p]
    xt = xpool.tile([P, NK * L * B], mybir.dt.bfloat16, tag="x")
    nc.sync.dma_start(xt[:], xpack[:])

    if repeat > 1 and HWLOOP:
        # For_i over repeat//UNROLL iterations of UNROLL bodies, plus the
        # remainder unrolled after the loop: exactly `repeat` bodies total.
        nfull, rem = divmod(repeat, UNROLL)
        if nfull > 0:
            with tc.For_i(0, nfull, 1, hint_engines=(
                    mybir.EngineType.PE, mybir.EngineType.SP)):
                for _ in range(UNROLL):
                    _emit_body(tc, xt, wpack, out, wpool, opool, pspool)
        for _ in range(rem):
            _emit_body(tc, xt, wpack, out, wpool, opool, pspool)
    else:
        for _ in range(repeat):
            _emit_body(tc, xt, wpack, out, wpool, opool, pspool)


def _emit_body(tc, xt, wpack, out, wpool, opool, pspool):
    nc = tc.nc
    for i in range(L):
        n = i + 1
        # One merged weight DMA per i covering both k tiles (adjacent in
        # wpack): [128, 2*n*768] bf16. wpack is partition-major, so each
        # partition row is one contiguous 2*n*1536B run.
        wt = wpool.tile([P, NK * n * A], mybir.dt.bfloat16, tag="w")
        base = _WBASE[(i, 0)] * A
        nc.sync.dma_start(wt[:], wpack[:, base:base + NK * n * A])
        # within wt, block (k, j) occupies columns [(k*n + j)*A : +A]
        jks = [(j, k) for k in range(NK) for j in range(n)]   # k-major
        acs = [(0, AC), (AC, AC)]
        ot = opool.tile([P, NB * A], mybir.dt.bfloat16)
        for bt in range(NB):
            pss = [pspool.tile([P, w], mybir.dt.float32, name=f"ps{ci}",
                               tag=f"ps{ci}", bufs=PSBUFS // 2)
                   for ci, (_, w) in enumerate(acs)]
            for ps, (off, w) in zip(pss, acs):
                for t, (j, k) in enumerate(jks):
                    nc.tensor.matmul(
                        ps[:],
                        xt[:, (k * L + j) * B + bt * P:(k * L + j) * B + bt * P + P],
                        wt[:, (k * n + j) * A + off:(k * n + j) * A + off + w],
                        start=(t == 0), stop=(t == len(jks) - 1),
                    )
            nc.vector.tensor_copy(ot[:, bt * A:bt * A + acs[0][1]], pss[0][:])
            nc.vector.tensor_copy(ot[:, bt * A + acs[0][1]:(bt + 1) * A], pss[1][:])
        # single out-DMA per i on the ACT HWDGE ring; out[i] holds both
        # batch tiles side by side: out[i, p, bt*A + a] = batch bt*128+p
        nc.scalar.dma_start(out[i, :, :], ot[:])


_NC_CACHE = {}


def build_module(repeat=1):
    key = (repeat, WBUFS, OBUFS, PSBUFS, HWLOOP, UNROLL)
    if key in _NC_CACHE:
        return _NC_CACHE[key]
    from contextlib import ExitStack
    nc = bacc.Bacc(
        "TRN2",
        target_bir_lowering=False,
        debug=False,
        enable_asserts=False,
        num_devices=NCORES,
    )
    xpack = nc.dram_tensor(
        "xpack", [P, NK * L * B], mybir.dt.bfloat16, kind="ExternalInput").ap()
    wpack = nc.dram_tensor(
        "wpack", [P, NPAIR * A], mybir.dt.bfloat16, kind="ExternalInput").ap()
    out = nc.dram_tensor(
        "out", [L, P, NB * A], mybir.dt.bfloat16, kind="ExternalOutput").ap()
    with tile.TileContext(nc) as tc:
        with ExitStack() as ctx:
            _emit_kernel(ctx, tc, xpack, wpack, out, repeat=repeat)
    nc.compile()
    _NC_CACHE[key] = nc
    return nc


def prep_inputs(x, W):
    """Build per-core packed inputs. Returns (xpacks[8], wpacks[8])."""
    # xpack[c][p, (k*L + j)*B + b] = x[b, j, c*FC + k*P + p]
    xb = np.asarray(x, dtype=BF16)                       # [256, 12, 2048]
    xr = xb.reshape(B, L, NCORES, NK, P).transpose(2, 4, 3, 1, 0)
    xpacks = np.ascontiguousarray(xr).reshape(NCORES, P, NK * L * B)

    # wpack[c]: partition-major; per (i, k) block occupies free columns
    # [_WBASE*A : (_WBASE+n)*A], j inner:
    #   wpack[c][p, (_WBASE[(i,k)] + j)*A + a] = W[i, j, c*FC + k*P + p, a]
    Ii = [i for i, j in _PAIRS]
    Jj = [j for i, j in _PAIRS]
    Wtri = np.asarray(W, dtype=BF16)[Ii, Jj]             # [78, 2048, 768]
    Wtri = Wtri.reshape(len(_PAIRS), NCORES, NK, P, A)   # [78, c, k, p, a]
    pidx = {}
    for t, (i, j) in enumerate(_PAIRS):
        pidx[(i, j)] = t
    sel_pair, sel_k = [], []
    for i in range(L):
        for k in range(NK):
            for j in range(i + 1):
                sel_pair.append(pidx[(i, j)])
                sel_k.append(k)
    Wp = Wtri[sel_pair, :, sel_k]                        # [156, c, 128, 768]
    Wp = np.ascontiguousarray(Wp.transpose(1, 2, 0, 3))  # [c, p, 156, a]
    wpacks = Wp.reshape(NCORES, P, NPAIR * A)
    return xpacks, wpacks


def run(x, W, trace=False, **kw):
    """Run the SPMD kernel; returns (full_output, BassKernelResults)."""
    x = np.asarray(x, dtype=np.float32)
    W = np.asarray(W, dtype=np.float32)
    xpacks, wpacks = prep_inputs(x, W)
    nc = build_module()
    in_maps = [{"xpack": xpacks[c], "wpack": wpacks[c]} for c in range(NCORES)]
    res = run_bass_kernel_spmd(nc, in_maps, list(range(NCORES)), trace=trace, **kw)
    total = res.results[0]["out"].astype(np.float32)
    for c in range(1, NCORES):
        total = total + res.results[c]["out"].astype(np.float32)
    # out[i, p, bt*A + a] = partial for batch bt*128+p -> [L, B, A]
    total = total.reshape(L, P, NB, A).transpose(0, 2, 1, 3).reshape(L, B, A)
    full = np.ascontiguousarray(total.transpose(1, 0, 2))
    return full, res


def kernel(x, W):
    full, _ = run(x, W)
    return full
